# revision 1
# baseline (speedup 1.0000x reference)
"""HB-LSTM cell fused Trainium2 kernel, data-parallel over 8 NeuronCores.

Computes, for gate order (f, i, o, u, k):
    pre  = x @ Wx[g].T + bx[g] + h_prev @ Uh[g].T + bh[g]
    f,i,o,u = sigmoid(pre[0..3]);  c = tanh(pre[4])
    kp = u*c + (1-u)*kp_prev
    k  = f*k_prev + i*kp
    h  = o*tanh(k)
Returns (h, k, kp), each [B, H] float32.

Sharding: batch dim B=65536 split across 8 cores (8192 rows each); weight
stacks replicated to every core.

Per-core structure (64 b-tiles of 128 rows):
  - x/h_prev loaded via SWDGE cast-DMA (fp32->bf16 in flight, Pool ring),
    staged c-major so ONE xbar DMA-transpose per (input, i-chunk, group)
    yields the feature-major lhsT tiles the PE needs.
  - 5-gate pre-activations accumulate in one [128,1280] PSUM tile per b-tile
    (12 bf16 matmuls + K=1 ones-matmul for part of the bias).
  - Sigmoid/Tanh on ACT straight out of PSUM; remaining bias via one fused
    DVE add; elementwise tail split between DVE and GPSIMD.
  - DMA issue spread across SP / ACT HWDGE rings and the Pool SWDGE ring.
"""

import contextlib

import numpy as np

import concourse.bacc as bacc
import concourse.mybir as mybir
from concourse import tile
from concourse.bass_utils import run_bass_kernel_spmd

N_CORES = 8
B = 65536
IN = 256
H = 256
G5 = 5
BL = B // N_CORES          # rows per core
NT = BL // 128             # 64 b-tiles per core
GROUP = 4                  # b-tiles per DMA group
NG = NT // GROUP
DG = G5 * H                # 1280 = all-gate column span
F32 = mybir.dt.float32
BF16 = mybir.dt.bfloat16
AF = mybir.ActivationFunctionType

# Gates [0, PE_BIAS_GATES) get bias from K=1 ones-matmuls on the PE; the rest
# from a fused DVE bias-add (balances PE vs DVE/ACT load). 0..5.
PE_BIAS_GATES = 2

# Engine for each elementwise op: "v" = DVE, "g" = GPSIMD.
OPS = {"d": "v", "e": "v", "kp": "v", "m": "v", "n": "v", "k": "v", "h": "v"}

# Bench mode: when set, the main loop runs LOOP_N times inside a hardware
# For_i loop so device time dominates RPC overhead in wall-clock.
LOOP_N = None

# Probe mode for HW decomposition benches: None = full kernel,
# "pe" = input loads + transposes + matmuls only (no ACT/DVE/stores).
PROBE = None

# x/h load path: "swdge" = cast-in-DMA on the Pool SWDGE ring;
# "hwdge" = fp32 HWDGE loads + GPSIMD tensor_copy cast to bf16.
LOAD_MODE = "swdge"

_CACHE = {}


def _build():
    if "nc" in _CACHE:
        return _CACHE["nc"]

    nc = bacc.Bacc("TRN2", target_bir_lowering=False, debug=False,
                   num_devices=N_CORES)

    x_d = nc.dram_tensor("x", [BL, IN], F32, kind="ExternalInput")
    h_d = nc.dram_tensor("h_prev", [BL, H], F32, kind="ExternalInput")
    k_d = nc.dram_tensor("k_prev", [BL, H], F32, kind="ExternalInput")
    kp_d = nc.dram_tensor("kp_prev", [BL, H], F32, kind="ExternalInput")
    wx_d = nc.dram_tensor("Wx", [G5, H, IN], F32, kind="ExternalInput")
    bx_d = nc.dram_tensor("bx", [G5, H], F32, kind="ExternalInput")
    uh_d = nc.dram_tensor("Uh", [G5, H, H], F32, kind="ExternalInput")
    bh_d = nc.dram_tensor("bh", [G5, H], F32, kind="ExternalInput")
    ho_d = nc.dram_tensor("h_out", [BL, H], F32, kind="ExternalOutput")
    ko_d = nc.dram_tensor("k_out", [BL, H], F32, kind="ExternalOutput")
    kpo_d = nc.dram_tensor("kp_out", [BL, H], F32, kind="ExternalOutput")

    vop = {"v": nc.vector, "g": nc.gpsimd}

    with tile.TileContext(nc) as tc:
        with tc.tile_pool(name="const", bufs=1) as cpool:
            # --- weights: fp32 -> bf16 (cast in DMA), i-major via xbar ---
            # WT[(side, c)]: [128 (i-chunk c), 1280 (g,h)] bf16 = matmul rhs
            WT = {}
            for side in ("x", "h"):
                for c in range(2):
                    WT[side, c] = cpool.tile([128, DG], BF16,
                                             name=f"WT_{side}{c}", tag=f"WT_{side}{c}")
            with tc.tile_pool(name="wload", bufs=2) as wload:
                for side, w_d in (("x", wx_d), ("h", uh_d)):
                    for g in range(G5):
                        w16 = wload.tile([128, 2, IN], BF16, tag="w16")
                        nc.gpsimd.dma_start(
                            w16[:],
                            w_d.ap()[g].rearrange("(hc p) i -> p hc i", p=128))
                        for c in range(2):
                            for hc in range(2):
                                col = g * H + hc * 128
                                nc.sync.dma_start(
                                    WT[side, c][:, col:col + 128],
                                    w16[:, hc, c * 128:(c + 1) * 128],
                                    transpose=True)

            # --- biases: bs16 [1,1280] bf16 row (PE path), biasb broadcast ---
            bs16 = cpool.tile([1, DG], BF16, tag="bs16")
            biasb = cpool.tile([128, DG], F32, tag="biasb")
            ones16 = cpool.tile([1, 128], BF16, tag="ones16")
            with tc.tile_pool(name="binit", bufs=1) as bpool, \
                 tc.tile_pool(name="binit_ps", bufs=1, space="PSUM") as bps:
                bxr = bpool.tile([G5, H], F32, tag="bxr")
                nc.sync.dma_start(bxr[:], bx_d.ap())
                bhr = bpool.tile([G5, H], F32, tag="bhr")
                nc.sync.dma_start(bhr[:], bh_d.ap())
                bsr = bpool.tile([G5, H], F32, tag="bsr")
                nc.vector.tensor_add(bsr[:], bxr[:], bhr[:])
                bsg = bpool.tile([G5, H], BF16, tag="bsg")
                nc.vector.tensor_copy(bsg[:], bsr[:])
                # flatten [5,256] -> one row [1,1280] (partition-major order)
                nc.sync.dma_start(bs16[:], bsg[:])
                nc.vector.memset(ones16[:], 1.0)
                psb = bps.tile([128, DG], F32, tag="psb")
                for n0 in range(0, DG, 512):
                    n1 = min(n0 + 512, DG)
                    nc.tensor.matmul(psb[:, n0:n1], ones16[:],
                                     bs16[:, n0:n1], start=True, stop=True)
                nc.vector.tensor_copy(biasb[:], psb[:])

            # --- main loop ---
            # c-major staging: [p, c, j(in group), q]
            x_cm = x_d.ap().rearrange("(n p) (c q) -> p c n q", p=128, q=128)
            h_cm = h_d.ap().rearrange("(n p) (c q) -> p c n q", p=128, q=128)
            k_t = k_d.ap().rearrange("(n p) i -> p n i", p=128)
            kp_t = kp_d.ap().rearrange("(n p) i -> p n i", p=128)
            ho_t = ho_d.ap().rearrange("(n p) i -> p n i", p=128)
            ko_t = ko_d.ap().rearrange("(n p) i -> p n i", p=128)
            kpo_t = kpo_d.ap().rearrange("(n p) i -> p n i", p=128)

            pe_cols = PE_BIAS_GATES * H
            loop_cm = (tc.For_i(0, LOOP_N, 1) if LOOP_N
                       else contextlib.nullcontext())
            with tc.tile_pool(name="io", bufs=2) as io, \
                 tc.tile_pool(name="work", bufs=4) as work, \
                 tc.tile_pool(name="psum", bufs=2, space="PSUM") as pp, \
                 loop_cm:
                for gi in range(NG):
                    nsl = slice(gi * GROUP, (gi + 1) * GROUP)
                    if PROBE == "mm":
                        # static zero activations: pure-PE probe
                        if "xTs" not in _CACHE:
                            _CACHE["xTs"] = cpool.tile([128, 2, GROUP, 128],
                                                       BF16, tag="xTs",
                                                       name="xTs")
                            _CACHE["hTs"] = cpool.tile([128, 2, GROUP, 128],
                                                       BF16, tag="hTs",
                                                       name="hTs")
                            nc.vector.memset(_CACHE["xTs"][:], 0.0)
                            nc.vector.memset(_CACHE["hTs"][:], 0.0)
                        xT = _CACHE["xTs"]
                        hT = _CACHE["hTs"]
                        for j in range(GROUP):
                            ps = pp.tile([128, DG], F32, tag="ps")
                            for n0 in range(0, pe_cols, 512):
                                n1 = min(n0 + 512, pe_cols)
                                nc.tensor.matmul(ps[:, n0:n1], ones16[:],
                                                 bs16[:, n0:n1],
                                                 start=True, stop=False)
                            for si, (side, aT) in enumerate((("x", xT),
                                                             ("h", hT))):
                                for c in range(2):
                                    lhsT = aT[:, c, j, :]
                                    for n0 in range(0, DG, 512):
                                        n1 = min(n0 + 512, DG)
                                        first = si == 0 and c == 0
                                        last = si == 1 and c == 1
                                        nc.tensor.matmul(
                                            ps[:, n0:n1], lhsT,
                                            WT[side, c][:, n0:n1],
                                            start=first and n0 >= pe_cols,
                                            stop=last)
                        continue
                    x16 = io.tile([128, 2, GROUP, 128], BF16, tag="x16")
                    h16 = io.tile([128, 2, GROUP, 128], BF16, tag="h16")
                    if LOAD_MODE == "swdge":
                        # cast-in-DMA loads (Pool SWDGE ring)
                        nc.gpsimd.dma_start(x16[:], x_cm[:, :, nsl, :])
                        nc.gpsimd.dma_start(h16[:], h_cm[:, :, nsl, :])
                    else:
                        # fp32 HWDGE loads + GPSIMD cast copies
                        x32 = io.tile([128, 2, GROUP, 128], F32, tag="x32")
                        nc.sync.dma_start(x32[:], x_cm[:, :, nsl, :])
                        h32 = io.tile([128, 2, GROUP, 128], F32, tag="h32")
                        nc.scalar.dma_start(h32[:], h_cm[:, :, nsl, :])
                        nc.gpsimd.tensor_copy(x16[:], x32[:])
                        nc.gpsimd.tensor_copy(h16[:], h32[:])
                    # fp32 state loads: kpr on SP ring, kppr on ACT ring
                    if PROBE != "pe":
                        kpr = io.tile([128, GROUP, H], F32, tag="kpr")
                        nc.sync.dma_start(kpr[:], k_t[:, nsl, :])
                        kppr = io.tile([128, GROUP, H], F32, tag="kppr")
                        nc.scalar.dma_start(kppr[:], kp_t[:, nsl, :])
                        kp_o = io.tile([128, GROUP, H], F32, tag="kp_o")
                        k_o = io.tile([128, GROUP, H], F32, tag="k_o")
                        h_o = io.tile([128, GROUP, H], F32, tag="h_o")

                    # batched xbar transposes (SP ring): one per (input, c)
                    xT = work.tile([128, 2, GROUP, 128], BF16, tag="xT")
                    hT = work.tile([128, 2, GROUP, 128], BF16, tag="hT")
                    for c in range(2):
                        nc.sync.dma_start(xT[:, c], x16[:, c], transpose=True)
                        nc.sync.dma_start(hT[:, c], h16[:, c], transpose=True)

                    for j in range(GROUP):
                        ps = pp.tile([128, DG], F32, tag="ps")
                        for n0 in range(0, pe_cols, 512):
                            n1 = min(n0 + 512, pe_cols)
                            nc.tensor.matmul(ps[:, n0:n1],
                                             ones16[:], bs16[:, n0:n1],
                                             start=True, stop=False)
                        for si, (side, aT) in enumerate((("x", xT), ("h", hT))):
                            for c in range(2):
                                lhsT = aT[:, c, j, :]
                                for n0 in range(0, DG, 512):
                                    n1 = min(n0 + 512, DG)
                                    first = si == 0 and c == 0
                                    last = si == 1 and c == 1
                                    nc.tensor.matmul(
                                        ps[:, n0:n1], lhsT,
                                        WT[side, c][:, n0:n1],
                                        start=first and n0 >= pe_cols,
                                        stop=last)

                        if PROBE == "pe":
                            continue
                        # activations; bias for gates >= PE_BIAS_GATES on DVE
                        if pe_cols >= 1024:
                            gates = work.tile([128, 1024], F32, tag="gates")
                            nc.scalar.activation(gates[:], ps[:, 0:1024],
                                                 AF.Sigmoid)
                            cg = work.tile([128, 256], F32, tag="cg")
                            if pe_cols >= DG:
                                nc.scalar.activation(cg[:], ps[:, 1024:DG],
                                                     AF.Tanh)
                            else:
                                pre = work.tile([128, 256], F32, tag="pre")
                                nc.vector.tensor_add(pre[:], ps[:, 1024:DG],
                                                     biasb[:, 1024:DG])
                                nc.scalar.activation(cg[:], pre[:], AF.Tanh)
                            f_ = gates[:, 0:256]
                            i_ = gates[:, 256:512]
                            o_ = gates[:, 512:768]
                            u_ = gates[:, 768:1024]
                        else:
                            fi = work.tile([128, pe_cols], F32, tag="fi")
                            nc.scalar.activation(fi[:], ps[:, 0:pe_cols],
                                                 AF.Sigmoid)
                            pre = work.tile([128, DG - pe_cols], F32, tag="pre")
                            nc.vector.tensor_add(pre[:], ps[:, pe_cols:DG],
                                                 biasb[:, pe_cols:DG])
                            ou = work.tile([128, 1024 - pe_cols], F32, tag="ou")
                            nc.scalar.activation(ou[:], pre[:, 0:1024 - pe_cols],
                                                 AF.Sigmoid)
                            cg = work.tile([128, 256], F32, tag="cg")
                            nc.scalar.activation(
                                cg[:], pre[:, 1024 - pe_cols:DG - pe_cols],
                                AF.Tanh)
                            f_ = fi[:, 0:256]
                            i_ = fi[:, 256:512]
                            o_ = ou[:, 512 - pe_cols:768 - pe_cols]
                            u_ = ou[:, 768 - pe_cols:1024 - pe_cols]

                        kpp_j = kppr[:, j, :]
                        kpr_j = kpr[:, j, :]

                        d = work.tile([128, 256], F32, tag="d")
                        vop[OPS["d"]].tensor_sub(d[:], cg[:], kpp_j)
                        e = work.tile([128, 256], F32, tag="e")
                        vop[OPS["e"]].tensor_mul(e[:], u_, d[:])
                        vop[OPS["kp"]].tensor_add(kp_o[:, j, :], e[:], kpp_j)
                        m = work.tile([128, 256], F32, tag="m")
                        vop[OPS["m"]].tensor_mul(m[:], f_, kpr_j)
                        n = work.tile([128, 256], F32, tag="n")
                        vop[OPS["n"]].tensor_mul(n[:], i_, kp_o[:, j, :])
                        vop[OPS["k"]].tensor_add(k_o[:, j, :], m[:], n[:])
                        tk = work.tile([128, 256], F32, tag="tk")
                        nc.scalar.activation(tk[:], k_o[:, j, :], AF.Tanh)
                        vop[OPS["h"]].tensor_mul(h_o[:, j, :], o_, tk[:])

                    # stores: k,kp on SP ring; h on ACT ring
                    if PROBE != "pe":
                        nc.sync.dma_start(kpo_t[:, nsl, :], kp_o[:])
                        nc.sync.dma_start(ko_t[:, nsl, :], k_o[:])
                        nc.scalar.dma_start(ho_t[:, nsl, :], h_o[:])

    nc.compile()
    _CACHE["nc"] = nc
    return nc


def kernel(x, h_prev, k_prev, kp_prev, Wx, bx, Uh, bh):
    x = np.asarray(x, dtype=np.float32)
    h_prev = np.asarray(h_prev, dtype=np.float32)
    k_prev = np.asarray(k_prev, dtype=np.float32)
    kp_prev = np.asarray(kp_prev, dtype=np.float32)
    Wx = np.ascontiguousarray(np.asarray(Wx, dtype=np.float32))
    bx = np.ascontiguousarray(np.asarray(bx, dtype=np.float32))
    Uh = np.ascontiguousarray(np.asarray(Uh, dtype=np.float32))
    bh = np.ascontiguousarray(np.asarray(bh, dtype=np.float32))

    nc = _build()
    in_maps = []
    for c in range(N_CORES):
        sl = slice(c * BL, (c + 1) * BL)
        in_maps.append({
            "x": np.ascontiguousarray(x[sl]),
            "h_prev": np.ascontiguousarray(h_prev[sl]),
            "k_prev": np.ascontiguousarray(k_prev[sl]),
            "kp_prev": np.ascontiguousarray(kp_prev[sl]),
            "Wx": Wx, "bx": bx, "Uh": Uh, "bh": bh,
        })
    res = run_bass_kernel_spmd(nc, in_maps, list(range(N_CORES)))
    h_out = np.concatenate([res.results[c]["h_out"] for c in range(N_CORES)], axis=0)
    k_out = np.concatenate([res.results[c]["k_out"] for c in range(N_CORES)], axis=0)
    kp_out = np.concatenate([res.results[c]["kp_out"] for c in range(N_CORES)], axis=0)
    return (h_out, k_out, kp_out)



# revision 4
# speedup vs baseline: 1.5602x; 1.5602x over previous
"""HB-LSTM cell fused Trainium2 kernel, data-parallel over 8 NeuronCores.

Computes, for gate order (f, i, o, u, k):
    pre  = x @ Wx[g].T + bx[g] + h_prev @ Uh[g].T + bh[g]
    f,i,o,u = sigmoid(pre[0..3]);  c = tanh(pre[4])
    kp = u*c + (1-u)*kp_prev
    k  = f*k_prev + i*kp
    h  = o*tanh(k)
Returns (h, k, kp), each [B, H] float32.

Sharding: batch dim B=65536 split across 8 cores (8192 rows each); weight
stacks replicated to every core.

Per-core structure (8 groups of 8 b-tiles of 128 rows):
  - x/h loaded via SWDGE cast-DMA (fp32->bf16 in flight) row-major, then ONE
    whole-group xbar DMA-transpose per input (>=4KB contiguous source) yields
    all 16 feature-major lhsT tiles for the group.
  - Per b-tile: 5-gate pre-activations accumulate in one [128,1280] PSUM
    tile: bias via K=1 ones-matmul (start), then 12 bf16 matmuls. The tanh
    gate's weights/bias are pre-scaled by 2 so that ONE sigmoid over all
    1280 cols yields the gates (tanh(x) = 2*sigmoid(2x)-1 fixed up on DVE).
  - Elementwise tail entirely in bf16 at group granularity (N=2048 per DVE
    op -> 2x perf mode, amortized op overhead); k_prev/kp_prev cast to bf16
    in the load DMA; outputs stored as bf16 (upcast to f32 on host).
"""

import contextlib

import numpy as np

import concourse.bacc as bacc
import concourse.mybir as mybir
from concourse import tile
from concourse.bass_utils import run_bass_kernel_spmd

N_CORES = 8
B = 65536
IN = 256
H = 256
G5 = 5
BL = B // N_CORES          # rows per core
NT = BL // 128             # 64 b-tiles per core
GROUP = 8                  # b-tiles per DMA group
NG = NT // GROUP
DG = G5 * H                # 1280 = all-gate column span
F32 = mybir.dt.float32
BF16 = mybir.dt.bfloat16
AF = mybir.ActivationFunctionType
ALU = mybir.AluOpType

# Bench mode: when set, the main loop runs LOOP_N times inside a hardware
# For_i loop so device time dominates RPC overhead in wall-clock.
LOOP_N = None

# Probe mode for HW decomposition benches: None = full kernel,
# "pe" = input loads + transposes + matmuls only (no ACT/DVE/stores),
# "mm" = matmuls only on static SBUF inputs.
PROBE = None

_CACHE = {}


def _build():
    if "nc" in _CACHE:
        return _CACHE["nc"]

    nc = bacc.Bacc("TRN2", target_bir_lowering=False, debug=False,
                   num_devices=N_CORES)

    x_d = nc.dram_tensor("x", [BL, IN], F32, kind="ExternalInput")
    h_d = nc.dram_tensor("h_prev", [BL, H], F32, kind="ExternalInput")
    k_d = nc.dram_tensor("k_prev", [BL, H], F32, kind="ExternalInput")
    kp_d = nc.dram_tensor("kp_prev", [BL, H], F32, kind="ExternalInput")
    wx_d = nc.dram_tensor("Wx", [G5, H, IN], F32, kind="ExternalInput")
    bx_d = nc.dram_tensor("bx", [G5, H], F32, kind="ExternalInput")
    uh_d = nc.dram_tensor("Uh", [G5, H, H], F32, kind="ExternalInput")
    bh_d = nc.dram_tensor("bh", [G5, H], F32, kind="ExternalInput")
    ho_d = nc.dram_tensor("h_out", [BL, H], BF16, kind="ExternalOutput")
    ko_d = nc.dram_tensor("k_out", [BL, H], BF16, kind="ExternalOutput")
    kpo_d = nc.dram_tensor("kp_out", [BL, H], BF16, kind="ExternalOutput")

    with tile.TileContext(nc) as tc:
        with tc.tile_pool(name="const", bufs=1) as cpool:
            # --- weights: fp32 -> bf16 (cast in DMA), i-major via xbar ---
            # WT[(side, c)]: [128 (i-chunk c), 1280 (g,h)] bf16 = matmul rhs
            # tanh-gate (g=4) weights pre-scaled by 2 (sigmoid folding).
            WT = {}
            for side in ("x", "h"):
                for c in range(2):
                    WT[side, c] = cpool.tile([128, DG], BF16,
                                             name=f"WT_{side}{c}", tag=f"WT_{side}{c}")
            with tc.tile_pool(name="wload", bufs=2) as wload:
                for side, w_d in (("x", wx_d), ("h", uh_d)):
                    for g in range(G5):
                        w16 = wload.tile([128, 2, IN], BF16, tag="w16")
                        nc.gpsimd.dma_start(
                            w16[:],
                            w_d.ap()[g].rearrange("(hc p) i -> p hc i", p=128))
                        if g == 4:
                            nc.vector.tensor_scalar_mul(w16[:], w16[:], 2.0)
                        for c in range(2):
                            for hc in range(2):
                                col = g * H + hc * 128
                                nc.sync.dma_start(
                                    WT[side, c][:, col:col + 128],
                                    w16[:, hc, c * 128:(c + 1) * 128],
                                    transpose=True)

            # --- bias row bs16 [1,1280] bf16 (tanh gate scaled by 2) ---
            bs16 = cpool.tile([1, DG], BF16, tag="bs16")
            ones16 = cpool.tile([1, 128], BF16, tag="ones16")
            with tc.tile_pool(name="binit", bufs=1) as bpool:
                bxr = bpool.tile([G5, H], F32, tag="bxr")
                nc.sync.dma_start(bxr[:], bx_d.ap())
                bhr = bpool.tile([G5, H], F32, tag="bhr")
                nc.sync.dma_start(bhr[:], bh_d.ap())
                bsr = bpool.tile([G5, H], F32, tag="bsr")
                nc.vector.tensor_add(bsr[:], bxr[:], bhr[:])
                bsg = bpool.tile([G5, H], BF16, tag="bsg")
                nc.vector.tensor_copy(bsg[:], bsr[:])
                # flatten [5,256] -> one row [1,1280] (partition-major order)
                nc.sync.dma_start(bs16[:], bsg[:])
                # tanh-gate (g=4) bias scaled by 2 (sigmoid folding)
                nc.vector.tensor_scalar_mul(bs16[:, 4 * H:], bs16[:, 4 * H:],
                                            2.0)
                nc.vector.memset(ones16[:], 1.0)

            # --- main loop ---
            # row-major staging: [p, n(tile in group), c, q]
            x_cm = x_d.ap().rearrange("(n p) (c q) -> p n c q", p=128, q=128)
            h_cm = h_d.ap().rearrange("(n p) (c q) -> p n c q", p=128, q=128)
            k_t = k_d.ap().rearrange("(n p) i -> p n i", p=128)
            kp_t = kp_d.ap().rearrange("(n p) i -> p n i", p=128)
            ho_t = ho_d.ap().rearrange("(n p) i -> p n i", p=128)
            ko_t = ko_d.ap().rearrange("(n p) i -> p n i", p=128)
            kpo_t = kpo_d.ap().rearrange("(n p) i -> p n i", p=128)

            loop_cm = (tc.For_i(0, LOOP_N, 1) if LOOP_N
                       else contextlib.nullcontext())
            with tc.tile_pool(name="io", bufs=2) as io, \
                 tc.tile_pool(name="work", bufs=2) as work, \
                 tc.tile_pool(name="psum", bufs=2, space="PSUM") as pp, \
                 loop_cm:
                if PROBE == "mm":
                    zT = cpool.tile([128, GROUP, 2, 128], BF16, tag="zT",
                                    name="zT")
                    nc.vector.memset(zT[:], 0.0)
                for gi in range(NG):
                    nsl = slice(gi * GROUP, (gi + 1) * GROUP)
                    if PROBE != "mm":
                        x16 = io.tile([128, GROUP, 2, 128], BF16, tag="x16")
                        h16 = io.tile([128, GROUP, 2, 128], BF16, tag="h16")
                        nc.gpsimd.dma_start(x16[:], x_cm[:, nsl])
                        nc.gpsimd.dma_start(h16[:], h_cm[:, nsl])
                        if PROBE != "pe":
                            kr = io.tile([128, GROUP, H], BF16, tag="kr")
                            nc.gpsimd.dma_start(kr[:], k_t[:, nsl, :])
                            kpp = io.tile([128, GROUP, H], BF16, tag="kpp")
                            nc.gpsimd.dma_start(kpp[:], kp_t[:, nsl, :])
                            kp_o = io.tile([128, GROUP, H], BF16, tag="kp_o")
                            k_o = io.tile([128, GROUP, H], BF16, tag="k_o")
                            h_o = io.tile([128, GROUP, H], BF16, tag="h_o")

                        # one whole-group xbar transpose per input:
                        # xT[:, j, c, :] = lhsT tile (feature-major)
                        xT = work.tile([128, GROUP, 2, 128], BF16, tag="xT")
                        hT = work.tile([128, GROUP, 2, 128], BF16, tag="hT")
                        nc.sync.dma_start(xT[:], x16[:], transpose=True)
                        nc.sync.dma_start(hT[:], h16[:], transpose=True)
                    else:
                        xT = zT
                        hT = zT

                    if PROBE != "pe" and PROBE != "mm":
                        gates = work.tile([128, GROUP, DG], BF16, tag="gates")

                    for j in range(GROUP):
                        ps = pp.tile([128, DG], F32, tag="ps")
                        for n0 in range(0, DG, 512):
                            n1 = min(n0 + 512, DG)
                            nc.tensor.matmul(ps[:, n0:n1], ones16[:],
                                             bs16[:, n0:n1],
                                             start=True, stop=False)
                        for si, (side, aT) in enumerate((("x", xT), ("h", hT))):
                            for c in range(2):
                                lhsT = aT[:, j, c, :]
                                last = si == 1 and c == 1
                                for n0 in range(0, DG, 512):
                                    n1 = min(n0 + 512, DG)
                                    nc.tensor.matmul(
                                        ps[:, n0:n1], lhsT,
                                        WT[side, c][:, n0:n1],
                                        start=False, stop=last)
                        if PROBE in ("pe", "mm"):
                            continue
                        # all 5 gates in one sigmoid (tanh gate pre-scaled)
                        nc.scalar.activation(gates[:, j, :], ps[:], AF.Sigmoid)

                    if PROBE in ("pe", "mm"):
                        continue

                    # ---- group elementwise tail, all bf16, N=GROUP*256 ----
                    f_ = gates[:, :, 0:256]
                    i_ = gates[:, :, 256:512]
                    o_ = gates[:, :, 512:768]
                    u_ = gates[:, :, 768:1024]
                    s4 = gates[:, :, 1024:1280]
                    c2 = work.tile([128, GROUP, H], BF16, tag="c2")
                    nc.vector.tensor_scalar(c2[:], s4, 2.0, -1.0,
                                            ALU.mult, ALU.add)
                    d = work.tile([128, GROUP, H], BF16, tag="d")
                    nc.vector.tensor_sub(d[:], c2[:], kpp[:])
                    e = work.tile([128, GROUP, H], BF16, tag="e")
                    nc.vector.tensor_mul(e[:], u_, d[:])
                    nc.vector.tensor_add(kp_o[:], e[:], kpp[:])
                    m = work.tile([128, GROUP, H], BF16, tag="m")
                    nc.vector.tensor_mul(m[:], f_, kr[:])
                    n = work.tile([128, GROUP, H], BF16, tag="n")
                    nc.vector.tensor_mul(n[:], i_, kp_o[:])
                    nc.vector.tensor_add(k_o[:], m[:], n[:])
                    tk = work.tile([128, GROUP, H], BF16, tag="tk")
                    nc.scalar.activation(tk[:], k_o[:], AF.Tanh)
                    nc.vector.tensor_mul(h_o[:], o_, tk[:])

                    # stores: k,kp on SP ring; h on ACT ring
                    nc.sync.dma_start(kpo_t[:, nsl, :], kp_o[:])
                    nc.sync.dma_start(ko_t[:, nsl, :], k_o[:])
                    nc.scalar.dma_start(ho_t[:, nsl, :], h_o[:])

    nc.compile()
    _CACHE["nc"] = nc
    return nc


def kernel(x, h_prev, k_prev, kp_prev, Wx, bx, Uh, bh):
    x = np.asarray(x, dtype=np.float32)
    h_prev = np.asarray(h_prev, dtype=np.float32)
    k_prev = np.asarray(k_prev, dtype=np.float32)
    kp_prev = np.asarray(kp_prev, dtype=np.float32)
    Wx = np.ascontiguousarray(np.asarray(Wx, dtype=np.float32))
    bx = np.ascontiguousarray(np.asarray(bx, dtype=np.float32))
    Uh = np.ascontiguousarray(np.asarray(Uh, dtype=np.float32))
    bh = np.ascontiguousarray(np.asarray(bh, dtype=np.float32))

    nc = _build()
    in_maps = []
    for c in range(N_CORES):
        sl = slice(c * BL, (c + 1) * BL)
        in_maps.append({
            "x": np.ascontiguousarray(x[sl]),
            "h_prev": np.ascontiguousarray(h_prev[sl]),
            "k_prev": np.ascontiguousarray(k_prev[sl]),
            "kp_prev": np.ascontiguousarray(kp_prev[sl]),
            "Wx": Wx, "bx": bx, "Uh": Uh, "bh": bh,
        })
    res = run_bass_kernel_spmd(nc, in_maps, list(range(N_CORES)))
    h_out = np.concatenate(
        [np.asarray(res.results[c]["h_out"]).astype(np.float32)
         for c in range(N_CORES)], axis=0)
    k_out = np.concatenate(
        [np.asarray(res.results[c]["k_out"]).astype(np.float32)
         for c in range(N_CORES)], axis=0)
    kp_out = np.concatenate(
        [np.asarray(res.results[c]["kp_out"]).astype(np.float32)
         for c in range(N_CORES)], axis=0)
    return (h_out, k_out, kp_out)


# revision 8
# speedup vs baseline: 1.5991x; 1.0249x over previous
"""HB-LSTM cell fused Trainium2 kernel, data-parallel over 8 NeuronCores.

Computes, for gate order (f, i, o, u, k):
    pre  = x @ Wx[g].T + bx[g] + h_prev @ Uh[g].T + bh[g]
    f,i,o,u = sigmoid(pre[0..3]);  c = tanh(pre[4])
    kp = u*c + (1-u)*kp_prev
    k  = f*k_prev + i*kp
    h  = o*tanh(k)
Returns (h, k, kp), each [B, H] float32.

Sharding: batch dim B=65536 split across 8 cores (8192 rows each); weight
stacks replicated to every core.

Per-core structure (8 groups of 8 b-tiles of 128 rows):
  - x/h loaded via SWDGE cast-DMA (fp32->bf16 in flight) row-major, then ONE
    whole-group xbar DMA-transpose per input (>=4KB contiguous source) yields
    all 16 feature-major lhsT tiles for the group.
  - Per b-tile: 5-gate pre-activations accumulate in one [128,1280] PSUM
    tile: bias via K=1 ones-matmul (start), then 12 bf16 matmuls. The tanh
    gate's weights/bias are pre-scaled by 2 so that ONE sigmoid over all
    1280 cols yields the gates (tanh(x) = 2*sigmoid(2x)-1 fixed up on DVE).
  - Elementwise tail entirely in bf16 at group granularity (N=2048 per DVE
    op -> 2x perf mode, amortized op overhead); k_prev/kp_prev cast to bf16
    in the load DMA; outputs stored as bf16 (upcast to f32 on host).
"""

import contextlib

import numpy as np

import concourse.bacc as bacc
import concourse.mybir as mybir
from concourse import tile
from concourse.bass_utils import run_bass_kernel_spmd

N_CORES = 8
B = 65536
IN = 256
H = 256
G5 = 5
BL = B // N_CORES          # rows per core
NT = BL // 128             # 64 b-tiles per core
GROUP = 8                  # b-tiles per DMA group
NG = NT // GROUP
DG = G5 * H                # 1280 = all-gate column span
F32 = mybir.dt.float32
BF16 = mybir.dt.bfloat16
F16 = mybir.dt.float16
DT = F16                   # compute dtype for GEMM inputs + elementwise tail
AF = mybir.ActivationFunctionType
ALU = mybir.AluOpType

# Bench mode: when set, the main loop runs LOOP_N times inside a hardware
# For_i loop so device time dominates RPC overhead in wall-clock.
LOOP_N = None

# Probe mode for HW decomposition benches: None = full kernel,
# "pe" = input loads + transposes + matmuls only (no ACT/DVE/stores),
# "mm" = matmuls only on static SBUF inputs,
# "lt" = input loads + transposes only (no PE/ACT/DVE/stores).
PROBE = None

# Which engine queue issues the three output stores.
STORE_ENGINE = "sync"

_CACHE = {}


def _build():
    if "nc" in _CACHE:
        return _CACHE["nc"]

    nc = bacc.Bacc("TRN2", target_bir_lowering=False, debug=False,
                   num_devices=N_CORES)

    x_d = nc.dram_tensor("x", [BL, IN], F32, kind="ExternalInput")
    h_d = nc.dram_tensor("h_prev", [BL, H], F32, kind="ExternalInput")
    k_d = nc.dram_tensor("k_prev", [BL, H], F32, kind="ExternalInput")
    kp_d = nc.dram_tensor("kp_prev", [BL, H], F32, kind="ExternalInput")
    wx_d = nc.dram_tensor("Wx", [G5, H, IN], F32, kind="ExternalInput")
    bx_d = nc.dram_tensor("bx", [G5, H], F32, kind="ExternalInput")
    uh_d = nc.dram_tensor("Uh", [G5, H, H], F32, kind="ExternalInput")
    bh_d = nc.dram_tensor("bh", [G5, H], F32, kind="ExternalInput")
    ho_d = nc.dram_tensor("h_out", [BL, H], DT, kind="ExternalOutput")
    ko_d = nc.dram_tensor("k_out", [BL, H], DT, kind="ExternalOutput")
    kpo_d = nc.dram_tensor("kp_out", [BL, H], DT, kind="ExternalOutput")

    with tile.TileContext(nc) as tc:
        with tc.tile_pool(name="const", bufs=1) as cpool:
            # --- weights: fp32 -> bf16 (cast in DMA), i-major via xbar ---
            # WT[(side, c)]: [128 (i-chunk c), 1280 (g,h)] bf16 = matmul rhs
            # tanh-gate (g=4) weights pre-scaled by 2 (sigmoid folding).
            WT = {}
            for side in ("x", "h"):
                for c in range(2):
                    WT[side, c] = cpool.tile([128, DG], DT,
                                             name=f"WT_{side}{c}", tag=f"WT_{side}{c}")
            with tc.tile_pool(name="wload", bufs=2) as wload:
                for side, w_d in (("x", wx_d), ("h", uh_d)):
                    for g in range(G5):
                        w16 = wload.tile([128, 2, IN], DT, tag="w16")
                        nc.gpsimd.dma_start(
                            w16[:],
                            w_d.ap()[g].rearrange("(hc p) i -> p hc i", p=128))
                        if g == 4:
                            nc.vector.tensor_scalar_mul(w16[:], w16[:], 2.0)
                        for c in range(2):
                            for hc in range(2):
                                col = g * H + hc * 128
                                nc.sync.dma_start(
                                    WT[side, c][:, col:col + 128],
                                    w16[:, hc, c * 128:(c + 1) * 128],
                                    transpose=True)

            # --- bias row bs16 [1,1280] bf16 (tanh gate scaled by 2) ---
            bs16 = cpool.tile([1, DG], DT, tag="bs16")
            ones16 = cpool.tile([1, 128], DT, tag="ones16")
            with tc.tile_pool(name="binit", bufs=1) as bpool:
                bxr = bpool.tile([G5, H], F32, tag="bxr")
                nc.sync.dma_start(bxr[:], bx_d.ap())
                bhr = bpool.tile([G5, H], F32, tag="bhr")
                nc.sync.dma_start(bhr[:], bh_d.ap())
                bsr = bpool.tile([G5, H], F32, tag="bsr")
                nc.vector.tensor_add(bsr[:], bxr[:], bhr[:])
                bsg = bpool.tile([G5, H], DT, tag="bsg")
                nc.vector.tensor_copy(bsg[:], bsr[:])
                # flatten [5,256] -> one row [1,1280] (partition-major order)
                nc.sync.dma_start(bs16[:], bsg[:])
                # tanh-gate (g=4) bias scaled by 2 (sigmoid folding)
                nc.vector.tensor_scalar_mul(bs16[:, 4 * H:], bs16[:, 4 * H:],
                                            2.0)
                nc.vector.memset(ones16[:], 1.0)

            # --- main loop ---
            # row-major staging: [p, n(tile in group), c, q]
            x_cm = x_d.ap().rearrange("(n p) (c q) -> p n c q", p=128, q=128)
            h_cm = h_d.ap().rearrange("(n p) (c q) -> p n c q", p=128, q=128)
            k_t = k_d.ap().rearrange("(n p) i -> p n i", p=128)
            kp_t = kp_d.ap().rearrange("(n p) i -> p n i", p=128)
            ho_t = ho_d.ap().rearrange("(n p) i -> p n i", p=128)
            ko_t = ko_d.ap().rearrange("(n p) i -> p n i", p=128)
            kpo_t = kpo_d.ap().rearrange("(n p) i -> p n i", p=128)

            loop_cm = (tc.For_i(0, LOOP_N, 1) if LOOP_N
                       else contextlib.nullcontext())
            with tc.tile_pool(name="io", bufs=2) as io, \
                 tc.tile_pool(name="work", bufs=2) as work, \
                 tc.tile_pool(name="psum", bufs=2, space="PSUM") as pp, \
                 loop_cm:
                if PROBE == "mm":
                    zT = cpool.tile([128, GROUP, 2, 128], DT, tag="zT",
                                    name="zT")
                    nc.vector.memset(zT[:], 0.0)
                for gi in range(NG):
                    nsl = slice(gi * GROUP, (gi + 1) * GROUP)
                    if PROBE != "mm":
                        x16 = io.tile([128, GROUP, 2, 128], DT, tag="x16")
                        h16 = io.tile([128, GROUP, 2, 128], DT, tag="h16")
                        nc.gpsimd.dma_start(x16[:], x_cm[:, nsl])
                        nc.gpsimd.dma_start(h16[:], h_cm[:, nsl])
                        if PROBE != "pe":
                            kr = io.tile([128, GROUP, H], DT, tag="kr")
                            nc.gpsimd.dma_start(kr[:], k_t[:, nsl, :])
                            kpp = io.tile([128, GROUP, H], DT, tag="kpp")
                            nc.gpsimd.dma_start(kpp[:], kp_t[:, nsl, :])
                            kp_o = io.tile([128, GROUP, H], DT, tag="kp_o")
                            k_o = io.tile([128, GROUP, H], DT, tag="k_o")
                            h_o = io.tile([128, GROUP, H], DT, tag="h_o")

                        # one whole-group xbar transpose per input:
                        # xT[:, j, c, :] = lhsT tile (feature-major)
                        xT = work.tile([128, GROUP, 2, 128], DT, tag="xT")
                        hT = work.tile([128, GROUP, 2, 128], DT, tag="hT")
                        nc.sync.dma_start(xT[:], x16[:], transpose=True)
                        nc.sync.dma_start(hT[:], h16[:], transpose=True)
                    else:
                        xT = zT
                        hT = zT
                    if PROBE == "lt":
                        continue

                    if PROBE != "pe" and PROBE != "mm":
                        gates = work.tile([128, GROUP, DG], DT, tag="gates")

                    for j in range(GROUP):
                        ps = pp.tile([128, DG], F32, tag="ps")
                        for n0 in range(0, DG, 512):
                            n1 = min(n0 + 512, DG)
                            nc.tensor.matmul(ps[:, n0:n1], ones16[:],
                                             bs16[:, n0:n1],
                                             start=True, stop=False)
                        for si, (side, aT) in enumerate((("x", xT), ("h", hT))):
                            for c in range(2):
                                lhsT = aT[:, j, c, :]
                                last = si == 1 and c == 1
                                for n0 in range(0, DG, 512):
                                    n1 = min(n0 + 512, DG)
                                    nc.tensor.matmul(
                                        ps[:, n0:n1], lhsT,
                                        WT[side, c][:, n0:n1],
                                        start=False, stop=last)
                        if PROBE in ("pe", "mm"):
                            continue
                        # all 5 gates in one sigmoid (tanh gate pre-scaled)
                        nc.scalar.activation(gates[:, j, :], ps[:], AF.Sigmoid)

                    if PROBE in ("pe", "mm"):
                        continue

                    # ---- group elementwise tail, all bf16, N=GROUP*256 ----
                    f_ = gates[:, :, 0:256]
                    i_ = gates[:, :, 256:512]
                    o_ = gates[:, :, 512:768]
                    u_ = gates[:, :, 768:1024]
                    s4 = gates[:, :, 1024:1280]
                    c2 = work.tile([128, GROUP, H], DT, tag="c2")
                    nc.vector.tensor_scalar(c2[:], s4, 2.0, -1.0,
                                            ALU.mult, ALU.add)
                    d = work.tile([128, GROUP, H], DT, tag="d")
                    nc.vector.tensor_sub(d[:], c2[:], kpp[:])
                    e = work.tile([128, GROUP, H], DT, tag="e")
                    nc.vector.tensor_mul(e[:], u_, d[:])
                    nc.vector.tensor_add(kp_o[:], e[:], kpp[:])
                    m = work.tile([128, GROUP, H], DT, tag="m")
                    nc.vector.tensor_mul(m[:], f_, kr[:])
                    n = work.tile([128, GROUP, H], DT, tag="n")
                    nc.vector.tensor_mul(n[:], i_, kp_o[:])
                    nc.vector.tensor_add(k_o[:], m[:], n[:])
                    tk = work.tile([128, GROUP, H], DT, tag="tk")
                    nc.scalar.activation(tk[:], k_o[:], AF.Tanh)
                    nc.vector.tensor_mul(h_o[:], o_, tk[:])

                    st = {"sync": nc.sync, "act": nc.scalar,
                          "pool": nc.gpsimd}[STORE_ENGINE]
                    st.dma_start(kpo_t[:, nsl, :], kp_o[:])
                    st.dma_start(ko_t[:, nsl, :], k_o[:])
                    nc.scalar.dma_start(ho_t[:, nsl, :], h_o[:])

    nc.compile()
    _CACHE["nc"] = nc
    return nc


def kernel(x, h_prev, k_prev, kp_prev, Wx, bx, Uh, bh):
    x = np.asarray(x, dtype=np.float32)
    h_prev = np.asarray(h_prev, dtype=np.float32)
    k_prev = np.asarray(k_prev, dtype=np.float32)
    kp_prev = np.asarray(kp_prev, dtype=np.float32)
    Wx = np.ascontiguousarray(np.asarray(Wx, dtype=np.float32))
    bx = np.ascontiguousarray(np.asarray(bx, dtype=np.float32))
    Uh = np.ascontiguousarray(np.asarray(Uh, dtype=np.float32))
    bh = np.ascontiguousarray(np.asarray(bh, dtype=np.float32))

    nc = _build()
    in_maps = []
    for c in range(N_CORES):
        sl = slice(c * BL, (c + 1) * BL)
        in_maps.append({
            "x": np.ascontiguousarray(x[sl]),
            "h_prev": np.ascontiguousarray(h_prev[sl]),
            "k_prev": np.ascontiguousarray(k_prev[sl]),
            "kp_prev": np.ascontiguousarray(kp_prev[sl]),
            "Wx": Wx, "bx": bx, "Uh": Uh, "bh": bh,
        })
    res = run_bass_kernel_spmd(nc, in_maps, list(range(N_CORES)))
    h_out = np.concatenate(
        [np.asarray(res.results[c]["h_out"]).astype(np.float32)
         for c in range(N_CORES)], axis=0)
    k_out = np.concatenate(
        [np.asarray(res.results[c]["k_out"]).astype(np.float32)
         for c in range(N_CORES)], axis=0)
    kp_out = np.concatenate(
        [np.asarray(res.results[c]["kp_out"]).astype(np.float32)
         for c in range(N_CORES)], axis=0)
    return (h_out, k_out, kp_out)
